# revision 2
# baseline (speedup 1.0000x reference)
"""Trainium2 Bass kernel v3 for sliding-window MHA (F5-TTS style) — bf16.

Sharding: 8 cores = 2 batches x 4 head-groups (4 heads / 256 inner cols per
core). All matmul operands bf16 (fp32 PSUM accumulation); rel-err budget
(2e-2) keeps ~10x margin.

Per-core layout is fully transposed [channels, tokens]:
  xt      [128, 8kb x N]    x^T resident in SBUF
  q/k     [128, 2 x N]      head-pairs stacked on partitions (h at 64*(h%2))
  v_sb    [128, 16kt x 260] per key-tile: 4 heads x (64 v-cols + ones col)
  aoT     [128, 2 x N]      attention out, inner-pair blocks
Phases:
  A: q/k w-stationary kb-outer passes (weights loaded once per kb, streamed
     over 4 token chunks; psum -> pack via activation bias-add);
     v x-stationary (output [tokens, inner] lands directly in v_sb layout).
  B: RoPE on head 0 (identity cos/sin on non-head-0 cores keeps SPMD uniform),
     overlapped with the v passes.
  C: per (t=128q, head-pair): additive band mask matmul (0/-240) + 3 scores
     matmuls into one PSUM tile -> Exp on scalar (psum -> bf16) -> 3 attn@v
     matmuls -> poT [q, 65] pairs whose col 64 is the softmax denominator ->
     paired reciprocal + per-partition tensor_scalar mult -> PE transpose of
     the head-pair stage -> vector copy into aoT.
  D: output projection row-slice after C; copies split scalar/vector, DMA on
     sync queue.
Host sums the 4 partial outputs per batch and adds bo.
"""
import os
import numpy as np
import ml_dtypes

BF16 = ml_dtypes.bfloat16

B, N, D = 2, 2048, 1024
H, HD = 16, 64
HPC = 4            # heads per core
SLICE = HPC * HD   # 256 inner cols per core
QT = 128           # query tile
ST = 384           # key strip width
KB = D // 128      # 8 contraction blocks
NT = N // QT       # 16 query tiles

_CACHE = {}
_last_results = None


def _strip_of(t):
    return min(max(t * QT - 128, 0), N - ST)


# ----------------------------------------------------------------------------
# device program
# ----------------------------------------------------------------------------
def _build_program(bv_nonzero: bool):
    import concourse.bacc as bacc
    import concourse.mybir as mybir
    import concourse.tile as tile
    from contextlib import ExitStack

    f32 = mybir.dt.float32
    bf16 = mybir.dt.bfloat16
    AF = mybir.ActivationFunctionType
    OP = mybir.AluOpType

    nc = bacc.Bacc("TRN2", target_bir_lowering=False, debug=False)

    xT_d = nc.dram_tensor("xT", [D, N], bf16, kind="ExternalInput").ap()
    wq_d = nc.dram_tensor("wq", [D, SLICE], bf16, kind="ExternalInput").ap()
    wk_d = nc.dram_tensor("wk", [D, SLICE], bf16, kind="ExternalInput").ap()
    wv_d = nc.dram_tensor("wv", [D, SLICE], bf16, kind="ExternalInput").ap()
    wo_d = nc.dram_tensor("wo", [SLICE, D], bf16, kind="ExternalInput").ap()
    bqk_d = nc.dram_tensor("bqk", [128, 4], f32, kind="ExternalInput").ap()
    bvr_d = nc.dram_tensor("bvrow", [1, SLICE], f32, kind="ExternalInput").ap()
    cos_d = nc.dram_tensor("cosT", [64, N], bf16, kind="ExternalInput").ap()
    sin_d = nc.dram_tensor("sinT", [64, N], bf16, kind="ExternalInput").ap()
    band_d = nc.dram_tensor("band", [128, 3 * QT], bf16, kind="ExternalInput").ap()
    id_d = nc.dram_tensor("ident", [128, 128], bf16, kind="ExternalInput").ap()
    out_d = nc.dram_tensor("out", [D, N], bf16, kind="ExternalOutput").ap()

    with tile.TileContext(nc) as tc:
        top = ExitStack()
        const = top.enter_context(tc.tile_pool(name="const", bufs=1))
        packs = top.enter_context(tc.tile_pool(name="packs", bufs=1))

        xt = packs.tile([128, KB * N], bf16, name="xt")
        q_pack = packs.tile([128, 2 * N], bf16, name="q_pack")
        k_pack = packs.tile([128, 2 * N], bf16, name="k_pack")
        v_sb = packs.tile([128, 16 * 260], bf16, name="v_sb")
        aoT = packs.tile([128, 2 * N], bf16, name="aoT")

        wq_t = const.tile([128, KB * SLICE], bf16, name="wq_t")
        wk_t = const.tile([128, KB * SLICE], bf16, name="wk_t")
        wv_t = const.tile([128, KB * SLICE], bf16, name="wv_t")
        wo_t = const.tile([128, 2 * D], bf16, name="wo_t")
        cos_t = const.tile([64, N], bf16, name="cos_t")
        sin_t = const.tile([64, N], bf16, name="sin_t")
        band_t = const.tile([128, 3 * QT], bf16, name="band_t")
        id_t = const.tile([128, 128], bf16, name="id_t")
        bqk_t = const.tile([128, 4], f32, name="bqk_t")

        # DMA order matters for startup: wq chunk 0 + xt chunk 0 first.
        for kb in range(KB):
            nc.sync.dma_start(
                wq_t[:, kb * SLICE:(kb + 1) * SLICE],
                wq_d[kb * 128:(kb + 1) * 128, :])
            for hf in range(2):
                eng = nc.scalar if hf == 0 else nc.sync
                eng.dma_start(
                    xt[:, kb * N + hf * 1024: kb * N + (hf + 1) * 1024],
                    xT_d[kb * 128:(kb + 1) * 128, hf * 1024:(hf + 1) * 1024])
        nc.sync.dma_start(
            wk_t[:].rearrange("p (b s) -> p b s", b=KB),
            wk_d[:].rearrange("(b p) s -> p b s", p=128))
        nc.sync.dma_start(
            wv_t[:].rearrange("p (b s) -> p b s", b=KB),
            wv_d[:].rearrange("(b p) s -> p b s", p=128))
        nc.sync.dma_start(cos_t[:], cos_d[:])
        nc.sync.dma_start(sin_t[:], sin_d[:])
        nc.sync.dma_start(band_t[:], band_d[:])
        nc.sync.dma_start(id_t[:], id_d[:])
        nc.sync.dma_start(bqk_t[:], bqk_d[:])
        nc.sync.dma_start(wo_t[:].rearrange("p (b s) -> p b s", b=2),
                          wo_d[:].rearrange("(b p) s -> p b s", p=128))

        # ones columns of v_sb (col 64 of each head block)
        ones_ap = v_sb[:].rearrange("p (t h e) -> p t h e", t=16, h=HPC)[:, :, :, 64:65]
        nc.vector.memset(ones_ap, 1.0)

        if bv_nonzero:
            bv_row = const.tile([1, SLICE], f32, name="bv_row")
            nc.sync.dma_start(bv_row[:], bvr_d[:])
            bv_bc = const.tile([128, SLICE], f32, name="bv_bc")
            nc.gpsimd.partition_broadcast(bv_bc[:], bv_row[0:1, :])

        # ---------------------------------------------------- phase A: q/k
        pa = ExitStack()
        ps_a = pa.enter_context(tc.tile_pool(name="ps_a", bufs=6, space="PSUM"))
        ps_v = pa.enter_context(tc.tile_pool(name="ps_v", bufs=2, space="PSUM"))

        rope = pa.enter_context(tc.tile_pool(name="rope", bufs=1))

        def emit_rope(pack):
            sw = rope.tile([64, N], bf16, tag="sw", name="sw")
            nc.sync.dma_start(sw[0:32, :], pack[32:64, 0:N])
            nc.sync.dma_start(sw[32:64, :], pack[0:32, 0:N])
            m = rope.tile([64, N], bf16, tag="m", name="m")
            nc.vector.tensor_tensor(m[:], sw[:], sin_t[:], OP.mult)
            t2 = rope.tile([64, N], bf16, tag="t2", name="t2")
            nc.vector.tensor_tensor(t2[:], pack[0:64, 0:N], cos_t[:], OP.mult)
            nc.vector.tensor_tensor(pack[0:64, 0:N], t2[:], m[:], OP.add)

        for wt, cb, dest, bcol in ((wq_t, 0, q_pack, 0), (wq_t, 1, q_pack, 1),
                                   (wk_t, 0, k_pack, 2), (wk_t, 1, k_pack, 3)):
            pss = [ps_a.tile([128, 512], f32, tag="ps_a", name=f"ps_a{i}")
                   for i in range(4)]
            for kb in range(KB):
                for ch in range(4):
                    nc.tensor.matmul(
                        pss[ch][:],
                        wt[:, kb * SLICE + cb * 128: kb * SLICE + (cb + 1) * 128],
                        xt[:, kb * N + ch * 512: kb * N + (ch + 1) * 512],
                        start=(kb == 0), stop=(kb == KB - 1))
            for ch in range(4):
                dst = dest[:, cb * N + ch * 512: cb * N + (ch + 1) * 512]
                if ch % 2 == 0:
                    nc.scalar.activation(dst, pss[ch][:], AF.Identity,
                                         bias=bqk_t[:, bcol:bcol + 1])
                else:
                    nc.vector.tensor_scalar_add(dst, pss[ch][:],
                                                bqk_t[:, bcol:bcol + 1])
            if (wt is wq_t and cb == 1):
                emit_rope(q_pack)
            if (wt is wk_t and cb == 1):
                emit_rope(k_pack)

        # ---------------------------------------------------- phase A: v
        for ti in range(16):
            pv = ps_v.tile([128, SLICE], f32, tag="ps_v", name="pv")
            for kb in range(KB):
                nc.tensor.matmul(
                    pv[:],
                    xt[:, kb * N + ti * 128: kb * N + (ti + 1) * 128],
                    wv_t[:, kb * SLICE:(kb + 1) * SLICE],
                    start=(kb == 0), stop=(kb == KB - 1))
            if bv_nonzero:
                nc.vector.tensor_tensor(pv[:], pv[:], bv_bc[:], OP.add)
            nc.vector.tensor_copy(
                v_sb[:, ti * 260:(ti + 1) * 260].rearrange(
                    "p (h e) -> p h e", h=HPC)[:, :, 0:64],
                pv[:].rearrange("p (h e) -> p h e", h=HPC))
        pa.close()

        # ---------------------------------------------------- phase C
        # kt-major scores: one k-load streams every query tile that sees it.
        pc = ExitStack()
        ps_s = pc.enter_context(tc.tile_pool(name="ps_s", bufs=3, space="PSUM"))
        ps_po = pc.enter_context(tc.tile_pool(name="ps_po", bufs=2, space="PSUM"))
        ps_tp = pc.enter_context(tc.tile_pool(name="ps_tp", bufs=1, space="PSUM"))
        expool = pc.enter_context(tc.tile_pool(name="expool", bufs=16))
        stg = pc.enter_context(tc.tile_pool(name="stg", bufs=3))
        rcp = pc.enter_context(tc.tile_pool(name="rcp", bufs=4))

        def qwin(kt):
            # query window seen by key tile kt: [qstart, qstart+W)
            qstart = max(0, (kt - 1) * QT)
            qend = min(N, (kt + 2) * QT)
            return qstart, qend - qstart

        exbs = {}

        def emit_kt(kt, h):
            qstart, W = qwin(kt)
            hb, hr = h // 2, h % 2
            ps = ps_s.tile([128, 384], f32, tag="ps_s", name="ps_s")
            nc.tensor.matmul(
                ps[:, 0:W],
                k_pack[64 * hr:64 * hr + 64, hb * N + kt * QT: hb * N + (kt + 1) * QT],
                q_pack[64 * hr:64 * hr + 64, hb * N + qstart: hb * N + qstart + W],
                start=True, stop=True)
            exb = expool.tile([128, 384], bf16, tag="exb", name="exb")
            nc.scalar.activation(exb[:, 0:W], ps[:, 0:W], AF.Exp, scale=0.125)
            # band offset: interior kt sees q starting at (kt-1)*128 -> col 0;
            # kt=0 window starts at q=0 -> pattern col 128
            boff = 128 if kt == 0 else 0
            nc.vector.tensor_tensor(exb[:, 0:W], exb[:, 0:W],
                                    band_t[:, boff:boff + W], OP.mult)
            exbs[(kt, h)] = exb

        def emit_pair_tail(t, hb):
            kts = [kt for kt in (t - 1, t, t + 1) if 0 <= kt < 16]
            po = ps_po.tile([128, 130], f32, tag="po", name="po")
            for hr in range(2):
                h = 2 * hb + hr
                for i, kt in enumerate(kts):
                    qstart, W = qwin(kt)
                    sl = t * QT - qstart
                    exb = exbs[(kt, h)]
                    nc.tensor.matmul(
                        po[:, hr * 65:(hr + 1) * 65],
                        exb[:, sl:sl + QT],
                        v_sb[:, kt * 260 + h * 65: kt * 260 + (h + 1) * 65],
                        start=(i == 0), stop=(i == len(kts) - 1))
            rc = rcp.tile([128, 2], f32, tag="rc", name="rc")
            nc.vector.reciprocal(
                rc[:].rearrange("p (h o) -> p h o", h=2),
                po[:].rearrange("p (h e) -> p h e", h=2)[:, :, 64:65])
            st = stg.tile([128, 128], bf16, tag="stg", name="stg")
            for hr in range(2):
                nc.vector.tensor_scalar_mul(
                    st[:, hr * 64:(hr + 1) * 64],
                    po[:, hr * 65: hr * 65 + 64], rc[:, hr:hr + 1])
            tp = ps_tp.tile([128, 128], bf16, tag="tp", name="tp")
            nc.tensor.transpose(tp[:], st[:], id_t[:])
            nc.scalar.copy(aoT[:, hb * N + t * QT: hb * N + (t + 1) * QT], tp[:])

        ps_w = pc.enter_context(tc.tile_pool(name="ps_w", bufs=2, space="PSUM"))
        obp = pc.enter_context(tc.tile_pool(name="obp", bufs=4))

        def emit_d(ch):
            for m in range(8):
                pw = ps_w.tile([128, 512], f32, tag="pw", name="pw")
                for icb in range(2):
                    nc.tensor.matmul(
                        pw[:],
                        wo_t[:, icb * D + m * 128: icb * D + (m + 1) * 128],
                        aoT[:, icb * N + ch * 512: icb * N + (ch + 1) * 512],
                        start=(icb == 0), stop=(icb == 1))
                ob = obp.tile([128, 512], bf16, tag="ob", name="ob")
                if m % 2 == 0:
                    nc.vector.tensor_copy(ob[:], pw[:])
                else:
                    nc.scalar.copy(ob[:], pw[:])
                nc.sync.dma_start(
                    out_d[m * 128:(m + 1) * 128, ch * 512:(ch + 1) * 512], ob[:])

        for kt in range(16):
            emit_kt(kt, 2); emit_kt(kt, 3)
            emit_kt(kt, 1); emit_kt(kt, 0)
            if kt >= 1:
                emit_pair_tail(kt - 1, 1)
                emit_pair_tail(kt - 1, 0)
            # D(ch) ready once tails of t = 4ch+3 are done (kt = 4ch+4)
            if kt >= 5 and (kt - 1) % 4 == 0:
                emit_d((kt - 5) // 4)
            for h in range(4):
                exbs.pop((kt - 3, h), None)
        emit_pair_tail(15, 1)
        emit_pair_tail(15, 0)
        emit_d(3)
        pc.close()

        # (phase D interleaved into C above)
        top.close()

    nc.compile()
    return nc


# ----------------------------------------------------------------------------
# host side
# ----------------------------------------------------------------------------
def _host_prep(x, freqs, Wq, bq, Wk, bk, Wv, bv, Wo, half):
    perm = np.concatenate([np.arange(0, 64, 2), np.arange(1, 64, 2)])
    cos_f = np.cos(freqs.astype(np.float64)).astype(np.float32)
    sin_f = np.sin(freqs.astype(np.float64)).astype(np.float32)
    cosT0 = np.ascontiguousarray(cos_f[:, perm].T)
    sinT0 = np.ascontiguousarray(sin_f[:, perm].T)
    sinT0[0:32] *= -1.0
    cos_id = np.ones((64, N), np.float32)
    sin_id = np.zeros((64, N), np.float32)

    # multiplicative band pattern in kt-window coords: key k = kt*128+p sees
    # query q = (kt-1)*128+j  =>  keep |p + 128 - j| <= half
    p = np.arange(128)
    j = np.arange(3 * QT)
    band = np.where(np.abs(p[:, None] + 128 - j[None, :]) <= half,
                    1.0, 0.0).astype(np.float32)

    ident = np.eye(128, dtype=np.float32)

    bv_any = bool(np.any(bv))
    maps = []
    for core in range(8):
        b, g = core // 4, core % 4
        sl = slice(g * SLICE, (g + 1) * SLICE)
        wq_s = np.ascontiguousarray(Wq[:, sl])
        wk_s = np.ascontiguousarray(Wk[:, sl])
        bq_s = bq[sl].copy()
        bk_s = bk[sl].copy()
        if g == 0:
            wq_s = wq_s.copy(); wq_s[:, 0:64] = wq_s[:, 0:64][:, perm]
            wk_s = wk_s.copy(); wk_s[:, 0:64] = wk_s[:, 0:64][:, perm]
            bq_s[0:64] = bq_s[0:64][perm]
            bk_s[0:64] = bk_s[0:64][perm]
            cosT, sinT = cosT0, sinT0
        else:
            cosT, sinT = cos_id, sin_id
        bqk = np.stack([bq_s[0:128], bq_s[128:256], bk_s[0:128], bk_s[128:256]],
                       axis=1).astype(np.float32)
        maps.append(dict(
            xT=np.ascontiguousarray(x[b].T).astype(BF16),
            wq=wq_s.astype(BF16), wk=wk_s.astype(BF16),
            wv=np.ascontiguousarray(Wv[:, sl]).astype(BF16),
            wo=np.ascontiguousarray(Wo[sl, :]).astype(BF16),
            bqk=bqk,
            bvrow=bv[sl][None, :].astype(np.float32),
            cosT=cosT.astype(BF16), sinT=sinT.astype(BF16),
            band=band.astype(BF16),
            ident=ident.astype(BF16),
        ))
    return maps, bv_any


def _numpy_fallback(x, mask, freqs, Wq, bq, Wk, bk, Wv, bv, Wo, bo, window_size):
    b, n, _ = x.shape
    h, hd = H, HD

    def rope(t):
        rot = freqs.shape[-1]
        tr = t[..., :rot].reshape(b, n, -1, 2)
        t1, t2 = tr[..., 0], tr[..., 1]
        rh = np.stack((-t2, t1), -1).reshape(b, n, rot)
        return np.concatenate(
            [t[..., :rot] * np.cos(freqs) + rh * np.sin(freqs), t[..., rot:]], -1)

    q = rope(x @ Wq + bq).reshape(b, n, h, hd).transpose(0, 2, 1, 3)
    k = rope(x @ Wk + bk).reshape(b, n, h, hd).transpose(0, 2, 1, 3)
    v = (x @ Wv + bv).reshape(b, n, h, hd).transpose(0, 2, 1, 3)
    i = np.arange(n)[:, None]
    j = np.arange(n)[None, :]
    half = int(window_size) // 2
    wm = (j >= i - half) & (j <= i + half)
    fm = wm[None, None] & mask[:, None, None, :]
    s = np.einsum("bhqd,bhkd->bhqk", q, k) / np.sqrt(np.float32(hd))
    s = np.where(fm, s, np.finfo(np.float32).min)
    s = s - s.max(-1, keepdims=True)
    e = np.exp(s)
    a = e / e.sum(-1, keepdims=True)
    out = np.einsum("bhqk,bhkd->bhqd", a, v).transpose(0, 2, 1, 3).reshape(b, n, h * hd)
    out = out @ Wo + bo
    return np.where(mask[..., None], out, 0.0).astype(np.float32)


def _ensure_ntff_hook():
    import sys
    import types
    try:
        from antenv.axon_hooks import get_axon_ntff_profile_hook  # noqa: F401
        return
    except ImportError:
        pass
    try:
        import antenv
        from trn_agent_boot.trn_boot import _ntff_profile_via_ctypes
        hook = _ntff_profile_via_ctypes("/opt/axon/libaxon_pjrt.so")
        mod = types.ModuleType("antenv.axon_hooks")
        mod.get_axon_ntff_profile_hook = lambda: hook
        mod.set_axon_ntff_profile_hook = lambda h: None
        sys.modules["antenv.axon_hooks"] = mod
        antenv.axon_hooks = mod
    except Exception:
        pass


def kernel(x, mask, freqs, Wq, bq, Wk, bk, Wv, bv, Wo, bo, window_size):
    global _last_results
    x = np.asarray(x, np.float32)
    mask_np = np.asarray(mask)
    freqs = np.asarray(freqs, np.float32)
    Wq = np.asarray(Wq, np.float32); Wk = np.asarray(Wk, np.float32)
    Wv = np.asarray(Wv, np.float32); Wo = np.asarray(Wo, np.float32)
    bq = np.asarray(bq, np.float32); bk = np.asarray(bk, np.float32)
    bv = np.asarray(bv, np.float32); bo = np.asarray(bo, np.float32)
    ws = int(window_size)

    if (x.shape != (B, N, D) or freqs.shape != (N, HD) or ws > 256 or ws % 2
            or not mask_np.all()):
        return _numpy_fallback(x, mask_np, freqs, Wq, bq, Wk, bk, Wv, bv, Wo, bo, ws)

    from concourse.bass_utils import run_bass_kernel_spmd

    maps, bv_any = _host_prep(x, freqs, Wq, bq, Wk, bk, Wv, bv, Wo, ws // 2)
    key = ("v7", bv_any)
    if key not in _CACHE:
        _CACHE[key] = _build_program(bv_any)
    nc = _CACHE[key]

    trace = bool(int(os.environ.get("KERNEL_TRACE", "0")))
    if trace:
        _ensure_ntff_hook()
    res = run_bass_kernel_spmd(nc, maps, core_ids=list(range(8)), trace=trace)
    _last_results = res

    out = np.empty((B, N, D), np.float32)
    for b in range(B):
        acc = res.results[4 * b]["out"].astype(np.float32)
        for g in range(1, 4):
            acc = acc + res.results[4 * b + g]["out"].astype(np.float32)
        out[b] = acc.T + bo[None, :]
    out *= mask_np[..., None].astype(np.float32)
    return out
